# revision 4
# baseline (speedup 1.0000x reference)
"""Trainium2 Bass kernel for ensemble CRPS loss.

Math (per (b,nt) pair, per (lat,lon) point, ens n=16):
  skill  = (1/n) sum_i |x_i - t|
  spread = (1/(n(n-1))) sum_{i!=j} |x_i - x_j|
  crps   = skill - spread/2

Using |a-b| = 2*max(a,b) - a - b and the Gini/rank identity
  sum_{i<j} |x_i - x_j| = 2*sum_{i<j} max(x_i,x_j) - (n-1)*sum_i x_i,
with K = sum_i max(x_i, t) and M = sum_{i<j} max(x_i, x_j), all the
sum_i x_i terms cancel exactly and

  crps_pt = K/8 - M/120 - t                       (n = 16)

The final scalar per (b,nt) is sum_{lat,lon} w[lat]*crps_pt / (nlat*nlon).

Device strategy (8 cores, data-parallel over the 32 (b,nt) pairs, 4 each):
  * Host passes, per core, an fp16 image of 25 "slots" of [128 lat, 4*256]:
    slots 0..15 = ensemble members, slot 16 = target, slots 17..24 = members
    0..7 duplicated.  With 17 logical elements (16 members + target), the
    cyclic shifts d=1..8 cover each of the C(17,2)=136 unordered pairs
    exactly once (17 is odd), so the WHOLE pairwise-max computation is
    8 uniform DVE tensor_tensor(max) ops (fp16 = 2x mode):
        maxd_d[:, i*1024:(i+1)*1024] = max(elem_i, elem_{(i+d) mod 17})
  * TensorE reduces every 1024-col position chunk over the lat axis with a
    lat-weight column as lhsT, accumulating into two PSUM rows:
        ps_A += (w/8)^T @ (x,t)-max chunks  and  (-w)^T @ t chunk
        ps_M += w^T @ (x,x)-max chunks
  * Host finishes: crps = (sum_lon ps_A - sum_lon ps_M / 120) / 32768,
    then the cumulative time mean.  Only [2,1024] f32 leaves each core.
"""

import os
import numpy as np

import concourse.bass as bass
import concourse.bacc as bacc
import concourse.tile as tile
from concourse import mybir
from concourse.bass_utils import run_bass_kernel_spmd

FP16 = mybir.dt.float16
FP32 = mybir.dt.float32

NCORES = 8
NLAT, NLON = 128, 256
ENS = 16
NPAIR = 4            # (b,nt) pairs per core
SLOT = NPAIR * NLON  # 1024 free elems per slot
NELEM = ENS + 1      # 16 members + target = 17 logical elements
NSLOT = NELEM + 8    # + 8 duplicated wrap slots
OPFD = NELEM * SLOT  # free size of one pairwise-max op

_CACHE = {}
LAST_RESULTS = None


def _build_program():
    nc = bacc.Bacc("TRN2", target_bir_lowering=False, debug=False,
                   num_devices=NCORES)

    xin = nc.dram_tensor("xin", [NLAT, NSLOT * SLOT], FP16,
                         kind="ExternalInput").ap()
    aux = nc.dram_tensor("aux", [NLAT, 3], FP16, kind="ExternalInput").ap()
    out = nc.dram_tensor("out", [1, 2 * SLOT], FP32, kind="ExternalOutput").ap()

    with tile.TileContext(nc) as tc:
        with tc.tile_pool(name="main", bufs=1) as main_pool, \
             tc.tile_pool(name="mx", bufs=3) as mx_pool, \
             tc.tile_pool(name="ps", bufs=1, space="PSUM") as ps_pool:

            t2 = main_pool.tile([NLAT, NSLOT * SLOT], FP16, tag="t2")
            auxt = main_pool.tile([NLAT, 3], FP16, tag="aux")
            outb = main_pool.tile([1, 2 * SLOT], FP32, tag="outb")

            ps_a = ps_pool.tile([1, SLOT], FP32, tag="psa")
            ps_m = ps_pool.tile([1, SLOT], FP32, tag="psm")

            # zero the PSUM accumulators (matmuls below never use start=True)
            nc.vector.memset(ps_a[:], 0.0)
            nc.vector.memset(ps_m[:], 0.0)

            nc.sync.dma_start(out=auxt[:], in_=aux)
            w_col = auxt[:, 0:1]    # w
            w8_col = auxt[:, 1:2]   # w/8
            mw_col = auxt[:, 2:3]   # -w

            # input image, chunked so compute can start before the tail lands
            chunks = [(0, 6), (6, 12), (12, 18), (18, NSLOT)]
            for s0, s1 in chunks:
                nc.sync.dma_start(out=t2[:, s0 * SLOT:s1 * SLOT],
                                  in_=xin[:, s0 * SLOT:s1 * SLOT])

            def emit_reduce(rhs_src, i, lhsT, ps):
                # one 1024-col position chunk -> two N=512 matmuls
                for h in range(2):
                    lo = i * SLOT + h * 512
                    nc.tensor.matmul(
                        ps[0:1, h * 512:(h + 1) * 512],
                        lhsT, rhs_src[:, lo:lo + 512],
                        start=False, stop=False, skip_group_check=True,
                    )

            # the lone -w^T @ t term
            emit_reduce(t2, ENS, mw_col, ps_a)

            for d in range(1, 9):
                mx = mx_pool.tile([NLAT, OPFD], FP16, tag="mx")
                nc.vector.tensor_tensor(
                    mx[:],
                    t2[:, 0:OPFD],
                    t2[:, d * SLOT:d * SLOT + OPFD],
                    mybir.AluOpType.max,
                )
                for i in range(NELEM):
                    if i == NELEM - 1 or i == NELEM - 1 - d:
                        emit_reduce(mx, i, w8_col, ps_a)   # (x, t) max
                    else:
                        emit_reduce(mx, i, w_col, ps_m)    # (x, x) max

            nc.scalar.copy(outb[0:1, 0:SLOT], ps_a[:])
            nc.scalar.copy(outb[0:1, SLOT:2 * SLOT], ps_m[:])
            nc.sync.dma_start(out=out, in_=outb[:])

    nc.compile()
    return nc


def _get_program():
    if "nc" not in _CACHE:
        _CACHE["nc"] = _build_program()
    return _CACHE["nc"]


def _prep_inputs(pred, target):
    pred = np.asarray(pred)
    target = np.asarray(target)
    b, ens, nt, nlat, nlon = pred.shape
    assert (b, ens, nt, nlat, nlon) == (2, ENS, 16, NLAT, NLON)

    # [(b,nt), ens, lat, lon]
    v = np.transpose(pred, (0, 2, 1, 3, 4)).reshape(b * nt, ens, nlat, nlon)
    tg = np.asarray(target).reshape(b * nt, nlat, nlon)

    xins = []
    for c in range(NCORES):
        vc = v[NPAIR * c:NPAIR * (c + 1)]          # [4, 16, 128, 256]
        tc = tg[NPAIR * c:NPAIR * (c + 1)]         # [4, 128, 256]
        mem = np.transpose(vc, (2, 1, 0, 3))       # [128, 16, 4, 256]
        tgt = np.transpose(tc, (1, 0, 2))[:, None]  # [128, 1, 4, 256]
        img = np.concatenate([mem, tgt, mem[:, 0:8]], axis=1)  # [128, 25, 4, 256]
        xins.append(np.ascontiguousarray(img).astype(np.float16)
                    .reshape(NLAT, NSLOT * SLOT))
    return xins


def kernel(pred, target, lat_weight):
    global LAST_RESULTS
    nc = _get_program()
    xins = _prep_inputs(pred, target)

    w = np.asarray(lat_weight).astype(np.float64)
    aux = np.stack([w, w / 8.0, -w], axis=1).astype(np.float16)  # [128, 3]

    in_maps = [{"xin": xins[c], "aux": aux} for c in range(NCORES)]
    res = run_bass_kernel_spmd(
        nc, in_maps, list(range(NCORES)),
        trace=bool(int(os.environ.get("CRPS_TRACE", "0"))),
    )
    LAST_RESULTS = res

    crps = np.empty(32, dtype=np.float64)
    for c in range(NCORES):
        o = res.results[c]["out"].astype(np.float64).reshape(2, SLOT)
        a = o[0].reshape(NPAIR, NLON).sum(axis=1)
        m = o[1].reshape(NPAIR, NLON).sum(axis=1)
        crps[NPAIR * c:NPAIR * (c + 1)] = (a - m / 120.0) / (NLAT * NLON)

    crps = crps.reshape(2, 16)
    denom = np.arange(1, 17, dtype=np.float64)
    out = np.cumsum(crps, axis=1) / denom
    return out.astype(np.float32)


# revision 5
# speedup vs baseline: 1.0182x; 1.0182x over previous
"""Trainium2 Bass kernel for ensemble CRPS loss.

Math (per (b,nt) pair, per (lat,lon) point, ens n=16):
  skill  = (1/n) sum_i |x_i - t|
  spread = (1/(n(n-1))) sum_{i!=j} |x_i - x_j|
  crps   = skill - spread/2

Using |a-b| = 2*max(a,b) - a - b and the Gini/rank identity
  sum_{i<j} |x_i - x_j| = 2*sum_{i<j} max(x_i,x_j) - (n-1)*sum_i x_i,
with K = sum_i max(x_i, t) and M = sum_{i<j} max(x_i, x_j), all the
sum_i x_i terms cancel exactly and

  crps_pt = K/8 - M/120 - t                       (n = 16)

The final scalar per (b,nt) is sum_{lat,lon} w[lat]*crps_pt / (nlat*nlon).

Device strategy (8 cores, data-parallel over the 32 (b,nt) pairs, 4 each):
  * Host passes, per core, an fp16 image of 25 "slots" of [128 lat, 4*256]:
    slots 0..15 = ensemble members, slot 16 = target, slots 17..24 = members
    0..7 duplicated.  With 17 logical elements (16 members + target), the
    cyclic shifts d=1..8 cover each of the C(17,2)=136 unordered pairs
    exactly once (17 is odd), so the WHOLE pairwise-max computation is
    8 uniform DVE tensor_tensor(max) ops (fp16 = 2x mode):
        maxd_d[:, i*1024:(i+1)*1024] = max(elem_i, elem_{(i+d) mod 17})
  * TensorE reduces every 1024-col position chunk over the lat axis with a
    lat-weight column as lhsT, accumulating into two PSUM rows:
        ps_A += (w/8)^T @ (x,t)-max chunks  and  (-w)^T @ t chunk
        ps_M += w^T @ (x,x)-max chunks
  * Host finishes: crps = (sum_lon ps_A - sum_lon ps_M / 120) / 32768,
    then the cumulative time mean.  Only [2,1024] f32 leaves each core.
"""

import os
import numpy as np

import concourse.bass as bass
import concourse.bacc as bacc
import concourse.tile as tile
from concourse import mybir
from concourse.bass_utils import run_bass_kernel_spmd

FP16 = mybir.dt.float16
FP32 = mybir.dt.float32

NCORES = 8
NLAT, NLON = 128, 256
ENS = 16
NPAIR = 4            # (b,nt) pairs per core
SLOT = NPAIR * NLON  # 1024 free elems per slot
NELEM = ENS + 1      # 16 members + target = 17 logical elements
NSLOT = NELEM + 8    # + 8 duplicated wrap slots
OPFD = NELEM * SLOT  # free size of one pairwise-max op

_CACHE = {}
LAST_RESULTS = None


def _build_program():
    nc = bacc.Bacc("TRN2", target_bir_lowering=False, debug=False,
                   num_devices=NCORES)

    xin = nc.dram_tensor("xin", [NLAT, NSLOT * SLOT], FP16,
                         kind="ExternalInput").ap()
    aux = nc.dram_tensor("aux", [NLAT, 3], FP16, kind="ExternalInput").ap()
    out = nc.dram_tensor("out", [1, 2 * SLOT], FP32, kind="ExternalOutput").ap()

    with tile.TileContext(nc) as tc:
        with tc.tile_pool(name="main", bufs=1) as main_pool, \
             tc.tile_pool(name="mx", bufs=3) as mx_pool, \
             tc.tile_pool(name="ps", bufs=1, space="PSUM") as ps_pool:

            t2 = main_pool.tile([NLAT, NSLOT * SLOT], FP16, tag="t2")
            auxt = main_pool.tile([NLAT, 3], FP16, tag="aux")
            outb = main_pool.tile([1, 2 * SLOT], FP32, tag="outb")

            ps_a = ps_pool.tile([1, SLOT], FP32, tag="psa")
            ps_m = ps_pool.tile([1, SLOT], FP32, tag="psm")

            # zero the PSUM accumulators (matmuls below never use start=True)
            nc.vector.memset(ps_a[:], 0.0)
            nc.vector.memset(ps_m[:], 0.0)

            nc.sync.dma_start(out=auxt[:], in_=aux)
            w_col = auxt[:, 0:1]    # w
            w8_col = auxt[:, 1:2]   # w/8
            mw_col = auxt[:, 2:3]   # -w

            # input image, chunked so compute can start before the tail lands
            chunks = [(0, 6), (6, 12), (12, 18), (18, NSLOT)]
            for s0, s1 in chunks:
                nc.sync.dma_start(out=t2[:, s0 * SLOT:s1 * SLOT],
                                  in_=xin[:, s0 * SLOT:s1 * SLOT])

            def emit_reduce(rhs_src, i, lhsT, ps):
                # one 1024-col position chunk -> two N=512 matmuls
                for h in range(2):
                    lo = i * SLOT + h * 512
                    nc.tensor.matmul(
                        ps[0:1, h * 512:(h + 1) * 512],
                        lhsT, rhs_src[:, lo:lo + 512],
                        start=False, stop=False, skip_group_check=True,
                    )

            # the lone -w^T @ t term
            emit_reduce(t2, ENS, mw_col, ps_a)

            for d in range(1, 9):
                mx = mx_pool.tile([NLAT, OPFD], FP16, tag="mx")
                nc.vector.tensor_tensor(
                    mx[:],
                    t2[:, 0:OPFD],
                    t2[:, d * SLOT:d * SLOT + OPFD],
                    mybir.AluOpType.max,
                )
                for i in range(NELEM):
                    if i == NELEM - 1 or i == NELEM - 1 - d:
                        emit_reduce(mx, i, w8_col, ps_a)   # (x, t) max
                    else:
                        emit_reduce(mx, i, w_col, ps_m)    # (x, x) max

            nc.scalar.copy(outb[0:1, 0:SLOT], ps_a[:])
            nc.scalar.copy(outb[0:1, SLOT:2 * SLOT], ps_m[:])
            nc.sync.dma_start(out=out, in_=outb[:])

    nc.compile()
    return nc


def _get_program():
    if "nc" not in _CACHE:
        _CACHE["nc"] = _build_program()
    return _CACHE["nc"]


def _prep_inputs(pred, target):
    pred = np.asarray(pred)
    target = np.asarray(target)
    b, ens, nt, nlat, nlon = pred.shape
    assert (b, ens, nt, nlat, nlon) == (2, ENS, 16, NLAT, NLON)

    # [(b,nt), ens, lat, lon]
    v = np.transpose(pred, (0, 2, 1, 3, 4)).reshape(b * nt, ens, nlat, nlon)
    tg = np.asarray(target).reshape(b * nt, nlat, nlon)

    xins = []
    for c in range(NCORES):
        vc = v[NPAIR * c:NPAIR * (c + 1)]          # [4, 16, 128, 256]
        tc = tg[NPAIR * c:NPAIR * (c + 1)]         # [4, 128, 256]
        mem = np.transpose(vc, (2, 1, 0, 3))       # [128, 16, 4, 256]
        tgt = np.transpose(tc, (1, 0, 2))[:, None]  # [128, 1, 4, 256]
        img = np.concatenate([mem, tgt, mem[:, 0:8]], axis=1)  # [128, 25, 4, 256]
        xins.append(np.ascontiguousarray(img).astype(np.float16)
                    .reshape(NLAT, NSLOT * SLOT))
    return xins


def kernel(pred, target, lat_weight):
    global LAST_RESULTS
    nc = _get_program()
    xins = _prep_inputs(pred, target)

    w = np.asarray(lat_weight).astype(np.float64)
    aux = np.stack([w, w / 8.0, -w], axis=1).astype(np.float16)  # [128, 3]

    in_maps = [{"xin": xins[c], "aux": aux} for c in range(NCORES)]
    res = run_bass_kernel_spmd(
        nc, in_maps, list(range(NCORES)),
        trace=bool(int(os.environ.get("CRPS_TRACE", "0"))),
        tmpdir=os.environ.get("CRPS_TRACE_DIR") or None,
    )
    LAST_RESULTS = res

    crps = np.empty(32, dtype=np.float64)
    for c in range(NCORES):
        o = res.results[c]["out"].astype(np.float64).reshape(2, SLOT)
        a = o[0].reshape(NPAIR, NLON).sum(axis=1)
        m = o[1].reshape(NPAIR, NLON).sum(axis=1)
        crps[NPAIR * c:NPAIR * (c + 1)] = (a - m / 120.0) / (NLAT * NLON)

    crps = crps.reshape(2, 16)
    denom = np.arange(1, 17, dtype=np.float64)
    out = np.cumsum(crps, axis=1) / denom
    return out.astype(np.float32)


# revision 9
# speedup vs baseline: 1.1024x; 1.0827x over previous
"""Trainium2 Bass kernel for ensemble CRPS loss.

Math (per (b,nt) pair, per (lat,lon) point, ens n=16):
  skill  = (1/n) sum_i |x_i - t|
  spread = (1/(n(n-1))) sum_{i!=j} |x_i - x_j|
  crps   = skill - spread/2

Using |a-b| = 2*max(a,b) - a - b and the Gini/rank identity
  sum_{i<j} |x_i - x_j| = 2*sum_{i<j} max(x_i,x_j) - (n-1)*sum_i x_i,
with K = sum_i max(x_i, t) and M = sum_{i<j} max(x_i, x_j), all the
sum_i x_i terms cancel exactly and

  crps_pt = K/8 - M/120 - t                       (n = 16)

The final scalar per (b,nt) is sum_{lat,lon} w[lat]*crps_pt / (nlat*nlon).

Device strategy (8 cores, data-parallel over the 32 (b,nt) pairs, 4 each):
  * Host passes, per core, an fp16 image of 25 "slots" of [128 lat, 4*256]:
    slots 0..15 = ensemble members, slot 16 = target, slots 17..24 = members
    0..7 duplicated.  With 17 logical elements (16 members + target), the
    cyclic shifts d=1..8 cover each of the C(17,2)=136 unordered pairs
    exactly once (17 is odd), so the WHOLE pairwise-max computation is
    8 uniform DVE tensor_tensor(max) ops (fp16 = 2x mode):
        maxd_d[:, i*1024:(i+1)*1024] = max(elem_i, elem_{(i+d) mod 17})
  * TensorE reduces every 1024-col position chunk over the lat axis with a
    lat-weight column as lhsT, accumulating into two PSUM rows:
        ps_A += (w/8)^T @ (x,t)-max chunks  and  (-w)^T @ t chunk
        ps_M += w^T @ (x,x)-max chunks
  * Host finishes: crps = (sum_lon ps_A - sum_lon ps_M / 120) / 32768,
    then the cumulative time mean.  Only [2,1024] f32 leaves each core.
"""

import os
import numpy as np

import concourse.bass as bass
import concourse.bacc as bacc
import concourse.tile as tile
from concourse import mybir
from concourse.bass_utils import run_bass_kernel_spmd

FP16 = mybir.dt.float16
FP32 = mybir.dt.float32

NCORES = 8
NLAT, NLON = 128, 256
ENS = 16
NPAIR = 4            # (b,nt) pairs per core
SLOT = NPAIR * NLON  # 1024 free elems per slot
NELEM = ENS + 1      # 16 members + target = 17 logical elements
NSLOT = NELEM + 8    # + 8 duplicated wrap slots
OPFD = NELEM * SLOT  # free size of one pairwise-max op

_CACHE = {}
LAST_RESULTS = None


def _build_program():
    nc = bacc.Bacc("TRN2", target_bir_lowering=False, debug=False,
                   num_devices=NCORES)

    xin = nc.dram_tensor("xin", [NLAT, NELEM * SLOT], FP16,
                         kind="ExternalInput").ap()
    aux = nc.dram_tensor("aux", [NLAT, 3], FP16, kind="ExternalInput").ap()
    out = nc.dram_tensor("out", [1, 2 * SLOT], FP32, kind="ExternalOutput").ap()

    with tile.TileContext(nc) as tc:
        with tc.tile_pool(name="main", bufs=1) as main_pool, \
             tc.tile_pool(name="mx", bufs=3) as mx_pool, \
             tc.tile_pool(name="ps", bufs=1, space="PSUM") as ps_pool:

            t2 = main_pool.tile([NLAT, NSLOT * SLOT], FP16, tag="t2")
            auxt = main_pool.tile([NLAT, 3], FP16, tag="aux")
            outb = main_pool.tile([1, 2 * SLOT], FP32, tag="outb")

            ps_a = ps_pool.tile([1, SLOT], FP32, tag="psa")
            ps_m = ps_pool.tile([1, SLOT], FP32, tag="psm")

            # zero the PSUM accumulators (matmuls below never use start=True)
            nc.vector.memset(ps_a[:], 0.0)
            nc.vector.memset(ps_m[:], 0.0)

            nc.sync.dma_start(out=auxt[:], in_=aux)
            w_col = auxt[:, 0:1]    # w
            w8_col = auxt[:, 1:2]   # w/8
            mw_col = auxt[:, 2:3]   # -w

            # input image (17 slots), chunked so compute starts early
            chunks = [(0, 4), (4, 8), (8, 12), (12, NELEM)]
            for s0, s1 in chunks:
                nc.sync.dma_start(out=t2[:, s0 * SLOT:s1 * SLOT],
                                  in_=xin[:, s0 * SLOT:s1 * SLOT])

            # duplicate slots 0..7 into the wrap region 17..24 on-device:
            # slot 17 via DVE (cheap, no ACT table-load latency), 18..24 via
            # the otherwise-idle ScalarE (also preloads the Copy table for
            # the PSUM evacuation at the end).
            nc.vector.tensor_copy(
                t2[:, NELEM * SLOT:(NELEM + 1) * SLOT], t2[:, 0:SLOT])
            nc.scalar.copy(
                t2[:, (NELEM + 1) * SLOT:NSLOT * SLOT], t2[:, SLOT:8 * SLOT])

            def emit_reduce(rhs_src, i, lhsT, ps):
                # one 1024-col position chunk -> two N=512 matmuls
                for h in range(2):
                    lo = i * SLOT + h * 512
                    nc.tensor.matmul(
                        ps[0:1, h * 512:(h + 1) * 512],
                        lhsT, rhs_src[:, lo:lo + 512],
                        start=False, stop=False, skip_group_check=True,
                    )

            # the lone -w^T @ t term
            emit_reduce(t2, ENS, mw_col, ps_a)

            # position-range sub-ops per shift d: early d's split so the DVE
            # starts as soon as the first DMA chunks land; the last d split
            # so the PE trail after the final DVE op is halved.
            splits = {1: [(0, 6), (6, 12), (12, 17)],
                      2: [(0, 9), (9, 17)],
                      8: [(0, 9), (9, 17)]}
            for d in range(1, 9):
                mx = mx_pool.tile([NLAT, OPFD], FP16, tag="mx")
                for i0, i1 in splits.get(d, [(0, 17)]):
                    nc.vector.tensor_tensor(
                        mx[:, i0 * SLOT:i1 * SLOT],
                        t2[:, i0 * SLOT:i1 * SLOT],
                        t2[:, (i0 + d) * SLOT:(i1 + d) * SLOT],
                        mybir.AluOpType.max,
                    )
                for i in range(NELEM):
                    if i == NELEM - 1 or i == NELEM - 1 - d:
                        emit_reduce(mx, i, w8_col, ps_a)   # (x, t) max
                    else:
                        emit_reduce(mx, i, w_col, ps_m)    # (x, x) max

            nc.scalar.copy(outb[0:1, 0:SLOT], ps_a[:])
            nc.scalar.copy(outb[0:1, SLOT:2 * SLOT], ps_m[:])
            nc.sync.dma_start(out=out, in_=outb[:])

    nc.compile()
    return nc


def _get_program():
    if "nc" not in _CACHE:
        _CACHE["nc"] = _build_program()
    return _CACHE["nc"]


def _prep_inputs(pred, target):
    pred = np.asarray(pred)
    target = np.asarray(target)
    b, ens, nt, nlat, nlon = pred.shape
    assert (b, ens, nt, nlat, nlon) == (2, ENS, 16, NLAT, NLON)

    # [(b,nt), ens, lat, lon]
    v = np.transpose(pred, (0, 2, 1, 3, 4)).reshape(b * nt, ens, nlat, nlon)
    tg = np.asarray(target).reshape(b * nt, nlat, nlon)

    xins = []
    for c in range(NCORES):
        vc = v[NPAIR * c:NPAIR * (c + 1)]          # [4, 16, 128, 256]
        tc = tg[NPAIR * c:NPAIR * (c + 1)]         # [4, 128, 256]
        mem = np.transpose(vc, (2, 1, 0, 3))       # [128, 16, 4, 256]
        tgt = np.transpose(tc, (1, 0, 2))[:, None]  # [128, 1, 4, 256]
        img = np.concatenate([mem, tgt], axis=1)  # [128, 17, 4, 256]
        xins.append(np.ascontiguousarray(img).astype(np.float16)
                    .reshape(NLAT, NELEM * SLOT))
    return xins


def kernel(pred, target, lat_weight):
    global LAST_RESULTS
    nc = _get_program()
    xins = _prep_inputs(pred, target)

    w = np.asarray(lat_weight).astype(np.float64)
    aux = np.stack([w, w / 8.0, -w], axis=1).astype(np.float16)  # [128, 3]

    in_maps = [{"xin": xins[c], "aux": aux} for c in range(NCORES)]
    res = run_bass_kernel_spmd(
        nc, in_maps, list(range(NCORES)),
        trace=bool(int(os.environ.get("CRPS_TRACE", "0"))),
        tmpdir=os.environ.get("CRPS_TRACE_DIR") or None,
    )
    LAST_RESULTS = res

    crps = np.empty(32, dtype=np.float64)
    for c in range(NCORES):
        o = res.results[c]["out"].astype(np.float64).reshape(2, SLOT)
        a = o[0].reshape(NPAIR, NLON).sum(axis=1)
        m = o[1].reshape(NPAIR, NLON).sum(axis=1)
        crps[NPAIR * c:NPAIR * (c + 1)] = (a - m / 120.0) / (NLAT * NLON)

    crps = crps.reshape(2, 16)
    denom = np.arange(1, 17, dtype=np.float64)
    out = np.cumsum(crps, axis=1) / denom
    return out.astype(np.float32)


# revision 12
# speedup vs baseline: 1.1147x; 1.0112x over previous
"""Trainium2 Bass kernel for ensemble CRPS loss.

Math (per (b,nt) pair, per (lat,lon) point, ens n=16):
  skill  = (1/n) sum_i |x_i - t|
  spread = (1/(n(n-1))) sum_{i!=j} |x_i - x_j|
  crps   = skill - spread/2

Using |a-b| = 2*max(a,b) - a - b and the Gini/rank identity
  sum_{i<j} |x_i - x_j| = 2*sum_{i<j} max(x_i,x_j) - (n-1)*sum_i x_i,
with K = sum_i max(x_i, t) and M = sum_{i<j} max(x_i, x_j), all the
sum_i x_i terms cancel exactly and

  crps_pt = K/8 - M/120 - t                       (n = 16)

The final scalar per (b,nt) is sum_{lat,lon} w[lat]*crps_pt / (nlat*nlon).

Device strategy (8 cores, data-parallel over the 32 (b,nt) pairs, 4 each):
  * Host passes, per core, an fp16 image of 25 "slots" of [128 lat, 4*256]:
    slots 0..15 = ensemble members, slot 16 = target, slots 17..24 = members
    0..7 duplicated.  With 17 logical elements (16 members + target), the
    cyclic shifts d=1..8 cover each of the C(17,2)=136 unordered pairs
    exactly once (17 is odd), so the WHOLE pairwise-max computation is
    8 uniform DVE tensor_tensor(max) ops (fp16 = 2x mode):
        maxd_d[:, i*1024:(i+1)*1024] = max(elem_i, elem_{(i+d) mod 17})
  * TensorE reduces every 1024-col position chunk over the lat axis with a
    lat-weight column as lhsT, accumulating into two PSUM rows:
        ps_A += (w/8)^T @ (x,t)-max chunks  and  (-w)^T @ t chunk
        ps_M += w^T @ (x,x)-max chunks
  * Host finishes: crps = (sum_lon ps_A - sum_lon ps_M / 120) / 32768,
    then the cumulative time mean.  Only [2,1024] f32 leaves each core.
"""

import os
import numpy as np

import concourse.bass as bass
import concourse.bacc as bacc
import concourse.tile as tile
from concourse import mybir
from concourse.bass_utils import run_bass_kernel_spmd

FP16 = mybir.dt.float16
FP32 = mybir.dt.float32

NCORES = 8
NLAT, NLON = 128, 256
ENS = 16
NPAIR = 4            # (b,nt) pairs per core
SLOT = NPAIR * NLON  # 1024 free elems per slot
NELEM = ENS + 1      # 16 members + target = 17 logical elements
NSLOT = NELEM + 8    # + 8 duplicated wrap slots
OPFD = NELEM * SLOT  # free size of one pairwise-max op

_CACHE = {}
LAST_RESULTS = None


def _build_program():
    nc = bacc.Bacc("TRN2", target_bir_lowering=False, debug=False,
                   num_devices=NCORES)

    xin = nc.dram_tensor("xin", [NLAT, NELEM * SLOT], FP16,
                         kind="ExternalInput").ap()
    aux = nc.dram_tensor("aux", [NLAT, 3], FP16, kind="ExternalInput").ap()
    out = nc.dram_tensor("out", [1, 2 * SLOT], FP32, kind="ExternalOutput").ap()

    with tile.TileContext(nc) as tc:
        with tc.tile_pool(name="main", bufs=1) as main_pool, \
             tc.tile_pool(name="mx", bufs=3) as mx_pool, \
             tc.tile_pool(name="ps", bufs=1, space="PSUM") as ps_pool:

            t2 = main_pool.tile([NLAT, NSLOT * SLOT], FP16, tag="t2")
            auxt = main_pool.tile([NLAT, 3], FP16, tag="aux")
            outb = main_pool.tile([1, 2 * SLOT], FP32, tag="outb")

            ps_a = ps_pool.tile([1, SLOT], FP32, tag="psa")
            ps_m = ps_pool.tile([1, SLOT], FP32, tag="psm")

            # zero the PSUM accumulators (matmuls below never use start=True)
            nc.vector.memset(ps_a[:], 0.0)
            nc.vector.memset(ps_m[:], 0.0)

            nc.sync.dma_start(out=auxt[:], in_=aux)
            w_col = auxt[:, 0:1]    # w
            w8_col = auxt[:, 1:2]   # w/8
            mw_col = auxt[:, 2:3]   # -w

            # input image (17 slots), chunked so compute starts early
            chunks = [(0, 4), (4, 8), (8, 12), (12, NELEM)]
            for s0, s1 in chunks:
                nc.sync.dma_start(out=t2[:, s0 * SLOT:s1 * SLOT],
                                  in_=xin[:, s0 * SLOT:s1 * SLOT])

            # duplicate slots 0..7 into the wrap region 17..24 on-device:
            # slot 17 via DVE (cheap, no ACT table-load latency), 18..24 via
            # the otherwise-idle ScalarE (also preloads the Copy table for
            # the PSUM evacuation at the end).
            nc.vector.tensor_copy(
                t2[:, NELEM * SLOT:(NELEM + 1) * SLOT], t2[:, 0:SLOT])
            nc.scalar.copy(
                t2[:, (NELEM + 1) * SLOT:NSLOT * SLOT], t2[:, SLOT:8 * SLOT])

            def emit_reduce(rhs_src, i, lhsT, ps):
                # one 1024-col position chunk -> two N=512 matmuls
                for h in range(2):
                    lo = i * SLOT + h * 512
                    nc.tensor.matmul(
                        ps[0:1, h * 512:(h + 1) * 512],
                        lhsT, rhs_src[:, lo:lo + 512],
                        start=False, stop=False, skip_group_check=True,
                    )

            # the lone -w^T @ t term
            emit_reduce(t2, ENS, mw_col, ps_a)

            # position-range sub-ops per shift d: early d's split so the DVE
            # starts as soon as the first DMA chunks land; the last d split
            # so the PE trail after the final DVE op is halved.
            splits = {1: [(0, 3), (3, 6), (6, 9), (9, 13), (13, 17)],
                      2: [(0, 6), (6, 12), (12, 17)],
                      3: [(0, 9), (9, 17)],
                      8: [(0, 9), (9, 14), (14, 17)]}
            for d in range(1, 9):
                mx = mx_pool.tile([NLAT, OPFD], FP16, tag="mx")
                for i0, i1 in splits.get(d, [(0, 17)]):
                    nc.vector.tensor_tensor(
                        mx[:, i0 * SLOT:i1 * SLOT],
                        t2[:, i0 * SLOT:i1 * SLOT],
                        t2[:, (i0 + d) * SLOT:(i1 + d) * SLOT],
                        mybir.AluOpType.max,
                    )
                for i in range(NELEM):
                    if i == NELEM - 1 or i == NELEM - 1 - d:
                        emit_reduce(mx, i, w8_col, ps_a)   # (x, t) max
                    else:
                        emit_reduce(mx, i, w_col, ps_m)    # (x, x) max

            nc.scalar.copy(outb[0:1, 0:SLOT], ps_a[:])
            nc.scalar.copy(outb[0:1, SLOT:2 * SLOT], ps_m[:])
            nc.sync.dma_start(out=out, in_=outb[:])

    nc.compile()
    return nc


def _get_program():
    if "nc" not in _CACHE:
        _CACHE["nc"] = _build_program()
    return _CACHE["nc"]


def _prep_inputs(pred, target):
    pred = np.asarray(pred)
    target = np.asarray(target)
    b, ens, nt, nlat, nlon = pred.shape
    assert (b, ens, nt, nlat, nlon) == (2, ENS, 16, NLAT, NLON)

    # [(b,nt), ens, lat, lon]
    v = np.transpose(pred, (0, 2, 1, 3, 4)).reshape(b * nt, ens, nlat, nlon)
    tg = np.asarray(target).reshape(b * nt, nlat, nlon)

    xins = []
    for c in range(NCORES):
        vc = v[NPAIR * c:NPAIR * (c + 1)]          # [4, 16, 128, 256]
        tc = tg[NPAIR * c:NPAIR * (c + 1)]         # [4, 128, 256]
        mem = np.transpose(vc, (2, 1, 0, 3))       # [128, 16, 4, 256]
        tgt = np.transpose(tc, (1, 0, 2))[:, None]  # [128, 1, 4, 256]
        img = np.concatenate([mem, tgt], axis=1)  # [128, 17, 4, 256]
        xins.append(np.ascontiguousarray(img).astype(np.float16)
                    .reshape(NLAT, NELEM * SLOT))
    return xins


def kernel(pred, target, lat_weight):
    global LAST_RESULTS
    nc = _get_program()
    xins = _prep_inputs(pred, target)

    w = np.asarray(lat_weight).astype(np.float64)
    aux = np.stack([w, w / 8.0, -w], axis=1).astype(np.float16)  # [128, 3]

    in_maps = [{"xin": xins[c], "aux": aux} for c in range(NCORES)]
    res = run_bass_kernel_spmd(
        nc, in_maps, list(range(NCORES)),
        trace=bool(int(os.environ.get("CRPS_TRACE", "0"))),
        tmpdir=os.environ.get("CRPS_TRACE_DIR") or None,
    )
    LAST_RESULTS = res

    crps = np.empty(32, dtype=np.float64)
    for c in range(NCORES):
        o = res.results[c]["out"].astype(np.float64).reshape(2, SLOT)
        a = o[0].reshape(NPAIR, NLON).sum(axis=1)
        m = o[1].reshape(NPAIR, NLON).sum(axis=1)
        crps[NPAIR * c:NPAIR * (c + 1)] = (a - m / 120.0) / (NLAT * NLON)

    crps = crps.reshape(2, 16)
    denom = np.arange(1, 17, dtype=np.float64)
    out = np.cumsum(crps, axis=1) / denom
    return out.astype(np.float32)
